# revision 4
# baseline (speedup 1.0000x reference)
"""Trainium2 Bass kernel for nn_AttentionSpikingNetwork (B=64, S=512).

Data-parallel over batch across 8 NeuronCores (8 batch elems per core).
All matmuls run as float32r (FP22, full PE rate) with exact hi/lo operand
splits (round-to-m11 hi + residual lo), giving fp32-class accuracy:
  - general x general: 3 passes (hi*hi + hi*lo + lo*hi)
  - spike (exact 0/1) x weight: 2 passes (s*hi + s*lo)
Activations flow transposed ([feat, seq]) so thresholds/biases are
per-partition; attention probabilities are transposed on the PE.
"""
import sys

sys.path.insert(0, "/opt/trn_rl_repo")

import numpy as np
from contextlib import ExitStack

import concourse.bass as bass
import concourse.bacc as bacc
import concourse.mybir as mybir
import concourse.tile as tile
from concourse.bass_utils import run_bass_kernel_spmd

F32 = mybir.dt.float32
F32R = mybir.dt.float32r
AF = mybir.ActivationFunctionType
OP = mybir.AluOpType

NCORES = 8
B, S, DIN, DEMB, DQK, DH2, DOUT = 64, 512, 784, 600, 64, 200, 10
NB = B // NCORES  # batch elems per core

def _chunks(total, step=128):
    return [(i, min(step, total - i)) for i in range(0, total, step)]

CH_DIN = _chunks(DIN)    # 7 chunks of <=128
CH_EMB = _chunks(DEMB)   # 5
CH_H2 = _chunks(DH2)     # 2
CH_S = _chunks(S)        # 4
CH_VN = [(0, 512), (512, 88)]  # V free-dim split (psum bank = 512 fp32)


def round_m11(a):
    """Round fp32 to 11 explicit mantissa bits (fp32r/FP22 grid), RNE."""
    a = np.ascontiguousarray(a, np.float32)
    u = a.view(np.uint32).astype(np.uint64)
    r = (u + 0x7FF + ((u >> 12) & 1)) & np.uint64(0xFFFFF000)
    return r.astype(np.uint32).view(np.float32)


def _split(a):
    hi = round_m11(a)
    lo = (a.astype(np.float32) - hi).astype(np.float32)
    return hi, lo


def build_nc(nb=NB):
    nc = bacc.Bacc()

    def par(name, shape, dt=F32R, out=False):
        return nc.declare_dram_parameter(name, list(shape), dt, isOutput=out)

    xh = par("xh", [nb, DIN, S])
    xl = par("xl", [nb, DIN, S])
    wEh = par("wEh", [DIN, DEMB]); wEl = par("wEl", [DIN, DEMB])
    wQh = par("wQh", [DEMB, DQK]); wQl = par("wQl", [DEMB, DQK])
    wKh = par("wKh", [DEMB, DQK]); wKl = par("wKl", [DEMB, DQK])
    wVh = par("wVh", [DEMB, DEMB]); wVl = par("wVl", [DEMB, DEMB])
    w2h = par("w2h", [DEMB, DH2]); w2l = par("w2l", [DEMB, DH2])
    w3h = par("w3h", [DH2, DOUT]); w3l = par("w3l", [DH2, DOUT])
    bE = par("bE", [DEMB, 1], F32); bQ = par("bQ", [DQK, 1], F32)
    bK = par("bK", [DQK, 1], F32); bV = par("bV", [DEMB, 1], F32)
    b2 = par("b2", [DH2, 1], F32); b3 = par("b3", [DOUT, 1], F32)
    ident = par("ident", [128, 128], F32)
    os_ = par("os", [nb, DOUT, S], F32, out=True)
    om_ = par("om", [nb, DOUT, S], F32, out=True)

    with ExitStack() as ctx:
        tc = ctx.enter_context(tile.TileContext(nc))
        wp = ctx.enter_context(tc.tile_pool(name="wp", bufs=1))
        xp = ctx.enter_context(tc.tile_pool(name="xp", bufs=2))
        sp = ctx.enter_context(tc.tile_pool(name="sp", bufs=1))
        small = ctx.enter_context(tc.tile_pool(name="small", bufs=2))
        outp = ctx.enter_context(tc.tile_pool(name="outp", bufs=1))
        ps_em = ctx.enter_context(tc.tile_pool(name="ps_em", bufs=1, space="PSUM"))
        ps = ctx.enter_context(tc.tile_pool(name="ps", bufs=3, space="PSUM"))

        # ---- resident weights / consts ----
        def wtiles(dram, chs, width, nm):
            hs = []
            for i, (c0, cn) in enumerate(chs):
                t = wp.tile([cn, width], F32R, name=f"{nm}{i}", tag=f"{nm}{i}")
                nc.sync.dma_start(out=t, in_=dram[c0:c0 + cn, :])
                hs.append(t)
            return hs

        wEh_t = wtiles(wEh, CH_DIN, DEMB, "wEh")
        wEl_t = wtiles(wEl, CH_DIN, DEMB, "wEl")
        wQh_t = wtiles(wQh, CH_EMB, DQK, "wQh"); wQl_t = wtiles(wQl, CH_EMB, DQK, "wQl")
        wKh_t = wtiles(wKh, CH_EMB, DQK, "wKh"); wKl_t = wtiles(wKl, CH_EMB, DQK, "wKl")
        wVh_t = wtiles(wVh, CH_EMB, DEMB, "wVh"); wVl_t = wtiles(wVl, CH_EMB, DEMB, "wVl")
        w2h_t = wtiles(w2h, CH_EMB, DH2, "w2h"); w2l_t = wtiles(w2l, CH_EMB, DH2, "w2l")
        w3h_t = wtiles(w3h, CH_H2, DOUT, "w3h"); w3l_t = wtiles(w3l, CH_H2, DOUT, "w3l")

        def btiles(dram, chs, nm):
            hs = []
            for i, (c0, cn) in enumerate(chs):
                t = wp.tile([cn, 1], F32, name=f"{nm}{i}", tag=f"{nm}{i}")
                nc.sync.dma_start(out=t, in_=dram[c0:c0 + cn, :])
                hs.append(t)
            return hs

        bE_t = btiles(bE, CH_EMB, "bE")
        bQ_t = btiles(bQ, [(0, DQK)], "bQ")[0]
        bK_t = btiles(bK, [(0, DQK)], "bK")[0]
        bV_t = btiles(bV, CH_EMB, "bV")
        b2_t = btiles(b2, CH_H2, "b2")
        b3_t = btiles(b3, [(0, DOUT)], "b3")[0]
        id_t = wp.tile([128, 128], F32, name="id_t", tag="id_t")
        nc.sync.dma_start(out=id_t, in_=ident[:, :])

        MM = nc.tensor.matmul

        for b in range(nb):
            # ============ embed.T = We @ x.T (k-outer, 3 passes) ============
            em_ps = []
            for i, (c0, cn) in enumerate(CH_EMB):
                t = ps_em.tile([cn, S], F32, name=f"em{i}", tag=f"em{i}")
                em_ps.append(t)
            nk = len(CH_DIN)
            for k, (k0, kn) in enumerate(CH_DIN):
                xh_t = xp.tile([kn, S], F32R, name="xh_t", tag="xh_t")
                xl_t = xp.tile([kn, S], F32R, name="xl_t", tag="xl_t")
                nc.sync.dma_start(out=xh_t, in_=xh[b, k0:k0 + kn, :])
                nc.sync.dma_start(out=xl_t, in_=xl[b, k0:k0 + kn, :])
                for i, (c0, cn) in enumerate(CH_EMB):
                    wh = wEh_t[k][:, c0:c0 + cn]
                    wl = wEl_t[k][:, c0:c0 + cn]
                    MM(em_ps[i], wh, xh_t, start=(k == 0), stop=False)
                    MM(em_ps[i], wh, xl_t, start=False, stop=False)
                    MM(em_ps[i], wl, xh_t, start=False, stop=(k == nk - 1))

            # spk1.T = (embed.T + bE > 0.5), exact 0/1 in f32r
            s1_t = []
            for i, (c0, cn) in enumerate(CH_EMB):
                t = sp.tile([cn, S], F32R, name=f"s1_{i}", tag=f"s1_{i}", bufs=2)
                nc.vector.tensor_scalar(t, em_ps[i], bE_t[i], 0.5, OP.add, OP.is_gt)
                s1_t.append(t)

            # ============ Q.T, K.T = Wq @ spk1.T + bq (2 passes) ============
            def qk(wh_t, wl_t, b_t, nm):
                q_ps = ps.tile([DQK, S], F32, name=f"{nm}_ps", tag="ps")
                n = len(CH_EMB)
                for i in range(n):
                    MM(q_ps, wh_t[i], s1_t[i], start=(i == 0), stop=False)
                    MM(q_ps, wl_t[i], s1_t[i], start=False, stop=(i == n - 1))
                qh_t = sp.tile([DQK, S], F32R, name=f"{nm}h", tag=f"{nm}h")
                ql_t = sp.tile([DQK, S], F32R, name=f"{nm}l", tag=f"{nm}l")
                # qh = round_m11(psum + bq); ql = (psum + bq) - qh  (exact split)
                nc.vector.tensor_scalar(qh_t, q_ps, b_t, None, OP.add)
                nc.vector.scalar_tensor_tensor(ql_t, q_ps, b_t,
                                               qh_t.bitcast(F32), OP.add,
                                               OP.subtract)
                return qh_t, ql_t

            qh_t, ql_t = qk(wQh_t, wQl_t, bQ_t, "q")
            kh_t, kl_t = qk(wKh_t, wKl_t, bK_t, "k")

            # ============ V natural = spk1 @ Wv.T (2 passes) ============
            vh_t, vl_t = [], []
            for ti, (t0, tn) in enumerate(CH_S):
                v_ps = [ps.tile([tn, w], F32, name=f"v_ps{j}", tag="ps")
                        for j, (v0, w) in enumerate(CH_VN)]
                n = len(CH_EMB)
                for i in range(n):
                    lh = s1_t[i][:, t0:t0 + tn]
                    for j, (v0, w) in enumerate(CH_VN):
                        MM(v_ps[j], lh, wVh_t[i][:, v0:v0 + w],
                           start=(i == 0), stop=False)
                        MM(v_ps[j], lh, wVl_t[i][:, v0:v0 + w],
                           start=False, stop=(i == n - 1))
                vh = sp.tile([tn, DEMB], F32R, name=f"vh{ti}", tag=f"vh{ti}")
                vl = sp.tile([tn, DEMB], F32R, name=f"vl{ti}", tag=f"vl{ti}")
                for j, (v0, w) in enumerate(CH_VN):
                    nc.vector.tensor_copy(vh[:, v0:v0 + w], v_ps[j])
                    nc.vector.scalar_tensor_tensor(
                        vl[:, v0:v0 + w], v_ps[j], 0.0,
                        vh[:, v0:v0 + w].bitcast(F32), OP.add, OP.subtract)
                vh_t.append(vh); vl_t.append(vl)

            # ============ scores + softmax (3 passes), per s-chunk ============
            p_t = []
            for si, (s0, sn) in enumerate(CH_S):
                sc_ps = ps.tile([sn, S], F32, name=f"sc_ps{si}", tag="ps")
                MM(sc_ps, qh_t[:, s0:s0 + sn], kh_t, start=True, stop=False)
                MM(sc_ps, qh_t[:, s0:s0 + sn], kl_t, start=False, stop=False)
                MM(sc_ps, ql_t[:, s0:s0 + sn], kh_t, start=False, stop=True)
                rowmax = small.tile([sn, 1], F32, name="rowmax", tag="rowmax")
                nc.vector.tensor_reduce(rowmax, sc_ps, mybir.AxisListType.X, OP.max)
                negmax = small.tile([sn, 1], F32, name="negmax", tag="negmax")
                nc.vector.tensor_scalar_mul(negmax, rowmax, -0.125)
                pt_ = sp.tile([sn, S], F32, name=f"p{si}", tag=f"p{si}")
                rowsum = small.tile([sn, 1], F32, name="rowsum", tag="rowsum")
                nc.scalar.activation(pt_, sc_ps, AF.Exp, bias=negmax, scale=0.125,
                                     accum_out=rowsum)
                invs = small.tile([sn, 1], F32, name="invs", tag="invs")
                nc.vector.reciprocal(invs, rowsum)
                nc.vector.tensor_scalar(pt_, pt_, invs, None, OP.mult)
                p_t.append(pt_)

            # ============ P.T via PE transpose + hi/lo split ============
            pth_t, ptl_t = [], []
            for ti, (t0, tn) in enumerate(CH_S):
                pt_ps = ps.tile([tn, S], F32, name=f"pt_ps{ti}", tag="ps")
                for si, (s0, sn) in enumerate(CH_S):
                    nc.tensor.transpose(pt_ps[:, s0:s0 + sn],
                                        p_t[si][:, t0:t0 + tn], id_t)
                ph = sp.tile([tn, S], F32R, name=f"pth{ti}", tag=f"pth{ti}")
                pl = sp.tile([tn, S], F32R, name=f"ptl{ti}", tag=f"ptl{ti}")
                nc.vector.tensor_copy(ph, pt_ps)
                nc.vector.scalar_tensor_tensor(pl, pt_ps, 0.0, ph.bitcast(F32),
                                               OP.add, OP.subtract)
                pth_t.append(ph); ptl_t.append(pl)

            # ============ attn_out.T = V.T @ P.T (3 passes) + bv + spk1.T ====
            s2h_t, s2l_t = [], []
            for i, (c0, cn) in enumerate(CH_EMB):
                ao_ps = ps.tile([cn, S], F32, name=f"ao_ps{i}", tag="ps")
                nt = len(CH_S)
                for ti in range(nt):
                    lh = vh_t[ti][:, c0:c0 + cn]
                    ll = vl_t[ti][:, c0:c0 + cn]
                    MM(ao_ps, lh, pth_t[ti], start=(ti == 0), stop=False)
                    MM(ao_ps, lh, ptl_t[ti], start=False, stop=False)
                    MM(ao_ps, ll, pth_t[ti], start=False, stop=(ti == nt - 1))
                raw = sp.tile([cn, S], F32, name="s2raw", tag="s2raw", bufs=2)
                # raw = (attn_out.T + bv) + spk1.T
                nc.scalar.activation(raw, ao_ps, AF.Identity, bias=bV_t[i])
                nc.vector.tensor_tensor(raw, raw, s1_t[i].bitcast(F32), OP.add)
                h = sp.tile([cn, S], F32R, name=f"s2h{i}", tag=f"s2h{i}")
                l = sp.tile([cn, S], F32R, name=f"s2l{i}", tag=f"s2l{i}")
                nc.vector.tensor_copy(h, raw)
                nc.vector.tensor_tensor(l, raw, h.bitcast(F32), OP.subtract)
                s2h_t.append(h); s2l_t.append(l)

            # ============ cur2.T = W2 @ spk2_in.T (3 passes), spk2 ============
            s2_t = []
            for hi, (h0, hn) in enumerate(CH_H2):
                c2_ps = ps.tile([hn, S], F32, name=f"c2_ps{hi}", tag="ps")
                n = len(CH_EMB)
                for i in range(n):
                    wh = w2h_t[i][:, h0:h0 + hn]
                    wl = w2l_t[i][:, h0:h0 + hn]
                    MM(c2_ps, wh, s2h_t[i], start=(i == 0), stop=False)
                    MM(c2_ps, wh, s2l_t[i], start=False, stop=False)
                    MM(c2_ps, wl, s2h_t[i], start=False, stop=(i == n - 1))
                t = sp.tile([hn, S], F32R, name=f"spk2_{hi}", tag=f"spk2_{hi}")
                nc.vector.tensor_scalar(t, c2_ps, b2_t[hi], 0.3, OP.add, OP.is_gt)
                s2_t.append(t)

            # ============ cur3.T = W3 @ spk2.T (2 passes), outputs ============
            c3_ps = ps.tile([DOUT, S], F32, name="c3_ps", tag="ps")
            n = len(CH_H2)
            for hi in range(n):
                MM(c3_ps, w3h_t[hi], s2_t[hi], start=(hi == 0), stop=False)
                MM(c3_ps, w3l_t[hi], s2_t[hi], start=False, stop=(hi == n - 1))
            spk3_t = outp.tile([DOUT, S], F32, name="spk3_t", tag="spk3_t")
            c3b_t = outp.tile([DOUT, S], F32, name="c3b_t", tag="c3b_t")
            mem3_t = outp.tile([DOUT, S], F32, name="mem3_t", tag="mem3_t")
            nc.vector.tensor_scalar(spk3_t, c3_ps, b3_t, 0.3, OP.add, OP.is_gt)
            nc.vector.tensor_scalar(c3b_t, c3_ps, b3_t, None, OP.add)
            nc.vector.scalar_tensor_tensor(mem3_t, spk3_t, -0.3, c3b_t,
                                           OP.mult, OP.add)
            nc.sync.dma_start(out=os_[b, :, :], in_=spk3_t)
            nc.sync.dma_start(out=om_[b, :, :], in_=mem3_t)

    nc.finalize()
    return nc


_NC_CACHE = {}


def _get_nc(nb):
    if nb not in _NC_CACHE:
        _NC_CACHE[nb] = build_nc(nb)
    return _NC_CACHE[nb]


def make_in_maps(x, We, be, Wq, bq, Wk, bk, Wv, bv, W2, b2, W3, b3,
                 ncores=NCORES):
    x = np.ascontiguousarray(x, np.float32)
    if x.max() > 1.0:
        x = (x * np.float32(1.0 / 255.0)).astype(np.float32)
    wEh, wEl = _split(np.ascontiguousarray(We.T))
    wQh, wQl = _split(np.ascontiguousarray(Wq.T))
    wKh, wKl = _split(np.ascontiguousarray(Wk.T))
    wVh, wVl = _split(np.ascontiguousarray(Wv.T))
    w2h, w2l = _split(np.ascontiguousarray(W2.T))
    w3h, w3l = _split(np.ascontiguousarray(W3.T))
    shared = dict(
        wEh=wEh, wEl=wEl, wQh=wQh, wQl=wQl, wKh=wKh, wKl=wKl,
        wVh=wVh, wVl=wVl, w2h=w2h, w2l=w2l, w3h=w3h, w3l=w3l,
        bE=np.ascontiguousarray(be.reshape(-1, 1), np.float32),
        bQ=np.ascontiguousarray(bq.reshape(-1, 1), np.float32),
        bK=np.ascontiguousarray(bk.reshape(-1, 1), np.float32),
        bV=np.ascontiguousarray(bv.reshape(-1, 1), np.float32),
        b2=np.ascontiguousarray(b2.reshape(-1, 1), np.float32),
        b3=np.ascontiguousarray(b3.reshape(-1, 1), np.float32),
        ident=np.eye(128, dtype=np.float32),
    )
    nb = x.shape[0] // ncores
    in_maps = []
    for c in range(ncores):
        xs = x[c * nb:(c + 1) * nb]                       # [nb, S, DIN]
        xT = np.ascontiguousarray(xs.transpose(0, 2, 1))  # [nb, DIN, S]
        xh_, xl_ = _split(xT)
        in_maps.append(dict(shared, xh=xh_, xl=xl_))
    return in_maps, nb


def kernel(x, We, be, Wq, bq, Wk, bk, Wv, bv, W2, b2, W3, b3, _trace=False):
    in_maps, nb = make_in_maps(x, We, be, Wq, bq, Wk, bk, Wv, bv, W2, b2, W3, b3)
    nc = _get_nc(nb)
    res = run_bass_kernel_spmd(nc, in_maps, list(range(NCORES)), trace=_trace)
    spk3 = np.concatenate([r["os"].transpose(0, 2, 1) for r in res.results], 0)
    mem3 = np.concatenate([r["om"].transpose(0, 2, 1) for r in res.results], 0)
    kernel.last_results = res
    return (np.ascontiguousarray(spk3, np.float32),
            np.ascontiguousarray(mem3, np.float32))


# revision 7
# speedup vs baseline: 1.0944x; 1.0944x over previous
"""Trainium2 Bass kernel for nn_AttentionSpikingNetwork (B=64, S=512).

Data-parallel over batch across 8 NeuronCores (8 batch elems per core).
All matmuls run as float32r (FP22, full PE rate) with exact hi/lo operand
splits (round-to-m11 hi + residual lo), giving fp32-class accuracy:
  - general x general: 3 passes (hi*hi + hi*lo + lo*hi)
  - spike (exact 0/1) x weight: 2 passes (s*hi + s*lo)
Activations flow transposed ([feat, seq]) so thresholds/biases are
per-partition; attention probabilities are transposed on the PE.
"""
import sys

sys.path.insert(0, "/opt/trn_rl_repo")

import numpy as np
from contextlib import ExitStack

import concourse.bass as bass
import concourse.bacc as bacc
import concourse.mybir as mybir
import concourse.tile as tile
from concourse.bass_utils import run_bass_kernel_spmd

F32 = mybir.dt.float32
F32R = mybir.dt.float32r
AF = mybir.ActivationFunctionType
OP = mybir.AluOpType

NCORES = 8
B, S, DIN, DEMB, DQK, DH2, DOUT = 64, 512, 784, 600, 64, 200, 10
NB = B // NCORES  # batch elems per core

def _chunks(total, step=128):
    return [(i, min(step, total - i)) for i in range(0, total, step)]

CH_DIN = _chunks(DIN)    # 7 chunks of <=128
CH_EMB = _chunks(DEMB)   # 5
CH_H2 = _chunks(DH2)     # 2
CH_S = _chunks(S)        # 4
CH_VN = [(0, 344), (344, 256)]  # V free-dim split; both >=256 keeps fp32r full-rate


def round_m11(a):
    """Round fp32 to 11 explicit mantissa bits (fp32r/FP22 grid), RNE."""
    a = np.ascontiguousarray(a, np.float32)
    u = a.view(np.uint32).astype(np.uint64)
    r = (u + 0x7FF + ((u >> 12) & 1)) & np.uint64(0xFFFFF000)
    return r.astype(np.uint32).view(np.float32)


def _split(a):
    hi = round_m11(a)
    lo = (a.astype(np.float32) - hi).astype(np.float32)
    return hi, lo


def build_nc(nb=NB):
    nc = bacc.Bacc()

    def par(name, shape, dt=F32R, out=False):
        return nc.declare_dram_parameter(name, list(shape), dt, isOutput=out)

    xh = par("xh", [nb, DIN, S])
    xl = par("xl", [nb, DIN, S])
    wEh = par("wEh", [DIN, DEMB]); wEl = par("wEl", [DIN, DEMB])
    wQh = par("wQh", [DEMB, DQK]); wQl = par("wQl", [DEMB, DQK])
    wKh = par("wKh", [DEMB, DQK]); wKl = par("wKl", [DEMB, DQK])
    wVh = par("wVh", [DEMB, DEMB]); wVl = par("wVl", [DEMB, DEMB])
    w2h = par("w2h", [DEMB, DH2]); w2l = par("w2l", [DEMB, DH2])
    w3h = par("w3h", [DH2, DOUT]); w3l = par("w3l", [DH2, DOUT])
    bE = par("bE", [DEMB, 1], F32); bQ = par("bQ", [DQK, 1], F32)
    bK = par("bK", [DQK, 1], F32); bV = par("bV", [DEMB, 1], F32)
    b2 = par("b2", [DH2, 1], F32); b3 = par("b3", [DOUT, 1], F32)
    ident = par("ident", [128, 128], F32)
    os_ = par("os", [nb, DOUT, S], F32, out=True)
    om_ = par("om", [nb, DOUT, S], F32, out=True)

    with ExitStack() as ctx:
        tc = ctx.enter_context(tile.TileContext(nc))
        wp = ctx.enter_context(tc.tile_pool(name="wp", bufs=1))
        xp = ctx.enter_context(tc.tile_pool(name="xp", bufs=2))
        sp = ctx.enter_context(tc.tile_pool(name="sp", bufs=1))
        small = ctx.enter_context(tc.tile_pool(name="small", bufs=2))
        outp = ctx.enter_context(tc.tile_pool(name="outp", bufs=1))
        ps_em = ctx.enter_context(tc.tile_pool(name="ps_em", bufs=1, space="PSUM"))
        ps = ctx.enter_context(tc.tile_pool(name="ps", bufs=3, space="PSUM"))

        # ---- resident weights / consts ----
        # DMA emission order is load order: the embed weights stream in
        # per-k-chunk interleaved with b=0's x chunks so the first matmul
        # starts after ~1MB, not after the full 8MB weight load. Everything
        # else loads during b=0's embed compute (see _load_rest below).
        def wtiles(dram, chs, width, nm, dma=True):
            hs = []
            for i, (c0, cn) in enumerate(chs):
                t = wp.tile([cn, width], F32R, name=f"{nm}{i}", tag=f"{nm}{i}")
                if dma:
                    nc.sync.dma_start(out=t, in_=dram[c0:c0 + cn, :])
                hs.append(t)
            return hs

        wEh_t = wtiles(wEh, CH_DIN, DEMB, "wEh", dma=False)
        wEl_t = wtiles(wEl, CH_DIN, DEMB, "wEl", dma=False)

        def btiles(dram, chs, nm):
            hs = []
            for i, (c0, cn) in enumerate(chs):
                t = wp.tile([cn, 1], F32, name=f"{nm}{i}", tag=f"{nm}{i}")
                nc.sync.dma_start(out=t, in_=dram[c0:c0 + cn, :])
                hs.append(t)
            return hs

        _rest = {}

        def _load_rest():
            _rest["wQh"] = wtiles(wQh, CH_EMB, DQK, "wQh")
            _rest["wQl"] = wtiles(wQl, CH_EMB, DQK, "wQl")
            _rest["wKh"] = wtiles(wKh, CH_EMB, DQK, "wKh")
            _rest["wKl"] = wtiles(wKl, CH_EMB, DQK, "wKl")
            _rest["bQ"] = btiles(bQ, [(0, DQK)], "bQ")[0]
            _rest["bK"] = btiles(bK, [(0, DQK)], "bK")[0]
            _rest["id"] = wp.tile([128, 128], F32, name="id_t", tag="id_t")
            nc.sync.dma_start(out=_rest["id"], in_=ident[:, :])
            _rest["wVh"] = wtiles(wVh, CH_EMB, DEMB, "wVh")
            _rest["wVl"] = wtiles(wVl, CH_EMB, DEMB, "wVl")
            _rest["bV"] = btiles(bV, CH_EMB, "bV")
            _rest["w2h"] = wtiles(w2h, CH_EMB, DH2, "w2h")
            _rest["w2l"] = wtiles(w2l, CH_EMB, DH2, "w2l")
            _rest["b2"] = btiles(b2, CH_H2, "b2")
            _rest["w3h"] = wtiles(w3h, CH_H2, DOUT, "w3h")
            _rest["w3l"] = wtiles(w3l, CH_H2, DOUT, "w3l")
            _rest["b3"] = btiles(b3, [(0, DOUT)], "b3")[0]

        bE_t = btiles(bE, CH_EMB, "bE")

        MM = nc.tensor.matmul

        for b in range(nb):
            # ============ embed.T = We @ x.T (k-outer, 3 passes) ============
            em_ps = []
            for i, (c0, cn) in enumerate(CH_EMB):
                t = ps_em.tile([cn, S], F32, name=f"em{i}", tag=f"em{i}")
                em_ps.append(t)
            nk = len(CH_DIN)
            for k, (k0, kn) in enumerate(CH_DIN):
                if b == 0:
                    nc.sync.dma_start(out=wEh_t[k], in_=wEh[k0:k0 + kn, :])
                    nc.sync.dma_start(out=wEl_t[k], in_=wEl[k0:k0 + kn, :])
                xh_t = xp.tile([kn, S], F32R, name="xh_t", tag="xh_t")
                xl_t = xp.tile([kn, S], F32R, name="xl_t", tag="xl_t")
                nc.sync.dma_start(out=xh_t, in_=xh[b, k0:k0 + kn, :])
                nc.sync.dma_start(out=xl_t, in_=xl[b, k0:k0 + kn, :])
                for i, (c0, cn) in enumerate(CH_EMB):
                    wh = wEh_t[k][:, c0:c0 + cn]
                    wl = wEl_t[k][:, c0:c0 + cn]
                    MM(em_ps[i], wh, xh_t, start=(k == 0), stop=False)
                    MM(em_ps[i], wh, xl_t, start=False, stop=False)
                    MM(em_ps[i], wl, xh_t, start=False, stop=(k == nk - 1))
            if b == 0:
                _load_rest()
                wQh_t, wQl_t = _rest["wQh"], _rest["wQl"]
                wKh_t, wKl_t = _rest["wKh"], _rest["wKl"]
                bQ_t, bK_t = _rest["bQ"], _rest["bK"]
                id_t = _rest["id"]
                wVh_t, wVl_t, bV_t = _rest["wVh"], _rest["wVl"], _rest["bV"]
                w2h_t, w2l_t, b2_t = _rest["w2h"], _rest["w2l"], _rest["b2"]
                w3h_t, w3l_t, b3_t = _rest["w3h"], _rest["w3l"], _rest["b3"]

            # spk1.T = (embed.T + bE > 0.5), exact 0/1 in f32r
            s1_t = []
            for i, (c0, cn) in enumerate(CH_EMB):
                t = sp.tile([cn, S], F32R, name=f"s1_{i}", tag=f"s1_{i}", bufs=2)
                nc.vector.tensor_scalar(t, em_ps[i], bE_t[i], 0.5, OP.add, OP.is_gt)
                s1_t.append(t)

            # ============ Q.T, K.T = Wq @ spk1.T + bq (2 passes) ============
            def qk(wh_t, wl_t, b_t, nm):
                q_ps = ps.tile([DQK, S], F32, name=f"{nm}_ps", tag="ps")
                n = len(CH_EMB)
                for i in range(n):
                    MM(q_ps, wh_t[i], s1_t[i], start=(i == 0), stop=False)
                    MM(q_ps, wl_t[i], s1_t[i], start=False, stop=(i == n - 1))
                qh_t = sp.tile([DQK, S], F32R, name=f"{nm}h", tag=f"{nm}h")
                ql_t = sp.tile([DQK, S], F32R, name=f"{nm}l", tag=f"{nm}l")
                # qh = round_m11(psum + bq); ql = (psum + bq) - qh  (exact split)
                nc.vector.tensor_scalar(qh_t, q_ps, b_t, None, OP.add)
                nc.vector.scalar_tensor_tensor(ql_t, q_ps, b_t,
                                               qh_t.bitcast(F32), OP.add,
                                               OP.subtract)
                return qh_t, ql_t

            qh_t, ql_t = qk(wQh_t, wQl_t, bQ_t, "q")
            kh_t, kl_t = qk(wKh_t, wKl_t, bK_t, "k")

            # ============ V natural = spk1 @ Wv.T (2 passes) ============
            vh_t, vl_t = [], []
            for ti, (t0, tn) in enumerate(CH_S):
                v_ps = [ps.tile([tn, w], F32, name=f"v_ps{j}", tag="ps")
                        for j, (v0, w) in enumerate(CH_VN)]
                n = len(CH_EMB)
                for i in range(n):
                    lh = s1_t[i][:, t0:t0 + tn]
                    for j, (v0, w) in enumerate(CH_VN):
                        MM(v_ps[j], lh, wVh_t[i][:, v0:v0 + w],
                           start=(i == 0), stop=False)
                        MM(v_ps[j], lh, wVl_t[i][:, v0:v0 + w],
                           start=False, stop=(i == n - 1))
                vh = sp.tile([tn, DEMB], F32R, name=f"vh{ti}", tag=f"vh{ti}")
                vl = sp.tile([tn, DEMB], F32R, name=f"vl{ti}", tag=f"vl{ti}")
                for j, (v0, w) in enumerate(CH_VN):
                    nc.vector.tensor_copy(vh[:, v0:v0 + w], v_ps[j])
                    nc.vector.scalar_tensor_tensor(
                        vl[:, v0:v0 + w], v_ps[j], 0.0,
                        vh[:, v0:v0 + w].bitcast(F32), OP.add, OP.subtract)
                vh_t.append(vh); vl_t.append(vl)

            # ============ scores + softmax (3 passes), per s-chunk ============
            p_t = []
            for si, (s0, sn) in enumerate(CH_S):
                sc_ps = ps.tile([sn, S], F32, name=f"sc_ps{si}", tag="ps")
                MM(sc_ps, qh_t[:, s0:s0 + sn], kh_t, start=True, stop=False)
                MM(sc_ps, qh_t[:, s0:s0 + sn], kl_t, start=False, stop=False)
                MM(sc_ps, ql_t[:, s0:s0 + sn], kh_t, start=False, stop=True)
                rowmax = small.tile([sn, 1], F32, name="rowmax", tag="rowmax")
                nc.vector.tensor_reduce(rowmax, sc_ps, mybir.AxisListType.X, OP.max)
                negmax = small.tile([sn, 1], F32, name="negmax", tag="negmax")
                nc.vector.tensor_scalar_mul(negmax, rowmax, -0.125)
                pt_ = sp.tile([sn, S], F32, name=f"p{si}", tag=f"p{si}")
                rowsum = small.tile([sn, 1], F32, name="rowsum", tag="rowsum")
                nc.scalar.activation(pt_, sc_ps, AF.Exp, bias=negmax, scale=0.125,
                                     accum_out=rowsum)
                invs = small.tile([sn, 1], F32, name="invs", tag="invs")
                nc.vector.reciprocal(invs, rowsum)
                nc.vector.tensor_scalar(pt_, pt_, invs, None, OP.mult)
                p_t.append(pt_)

            # ============ P.T via PE transpose + hi/lo split ============
            pth_t, ptl_t = [], []
            for ti, (t0, tn) in enumerate(CH_S):
                pt_ps = ps.tile([tn, S], F32, name=f"pt_ps{ti}", tag="ps")
                for si, (s0, sn) in enumerate(CH_S):
                    nc.tensor.transpose(pt_ps[:, s0:s0 + sn],
                                        p_t[si][:, t0:t0 + tn], id_t)
                ph = sp.tile([tn, S], F32R, name=f"pth{ti}", tag=f"pth{ti}")
                pl = sp.tile([tn, S], F32R, name=f"ptl{ti}", tag=f"ptl{ti}")
                nc.vector.tensor_copy(ph, pt_ps)
                nc.vector.scalar_tensor_tensor(pl, pt_ps, 0.0, ph.bitcast(F32),
                                               OP.add, OP.subtract)
                pth_t.append(ph); ptl_t.append(pl)

            # ============ attn_out.T = V.T @ P.T (3 passes) + bv + spk1.T ====
            s2h_t, s2l_t = [], []
            for i, (c0, cn) in enumerate(CH_EMB):
                ao_ps = ps.tile([cn, S], F32, name=f"ao_ps{i}", tag="ps")
                nt = len(CH_S)
                for ti in range(nt):
                    lh = vh_t[ti][:, c0:c0 + cn]
                    ll = vl_t[ti][:, c0:c0 + cn]
                    MM(ao_ps, lh, pth_t[ti], start=(ti == 0), stop=False)
                    MM(ao_ps, lh, ptl_t[ti], start=False, stop=False)
                    MM(ao_ps, ll, pth_t[ti], start=False, stop=(ti == nt - 1))
                raw = sp.tile([cn, S], F32, name="s2raw", tag="s2raw", bufs=2)
                # raw = (attn_out.T + bv) + spk1.T
                nc.scalar.activation(raw, ao_ps, AF.Identity, bias=bV_t[i])
                nc.vector.tensor_tensor(raw, raw, s1_t[i].bitcast(F32), OP.add)
                h = sp.tile([cn, S], F32R, name=f"s2h{i}", tag=f"s2h{i}")
                l = sp.tile([cn, S], F32R, name=f"s2l{i}", tag=f"s2l{i}")
                nc.vector.tensor_copy(h, raw)
                nc.vector.tensor_tensor(l, raw, h.bitcast(F32), OP.subtract)
                s2h_t.append(h); s2l_t.append(l)

            # ============ cur2.T = W2 @ spk2_in.T (3 passes), spk2 ============
            s2_t = []
            for hi, (h0, hn) in enumerate(CH_H2):
                c2_ps = ps.tile([hn, S], F32, name=f"c2_ps{hi}", tag="ps")
                n = len(CH_EMB)
                for i in range(n):
                    wh = w2h_t[i][:, h0:h0 + hn]
                    wl = w2l_t[i][:, h0:h0 + hn]
                    MM(c2_ps, wh, s2h_t[i], start=(i == 0), stop=False)
                    MM(c2_ps, wh, s2l_t[i], start=False, stop=False)
                    MM(c2_ps, wl, s2h_t[i], start=False, stop=(i == n - 1))
                t = sp.tile([hn, S], F32R, name=f"spk2_{hi}", tag=f"spk2_{hi}")
                nc.vector.tensor_scalar(t, c2_ps, b2_t[hi], 0.3, OP.add, OP.is_gt)
                s2_t.append(t)

            # ============ cur3.T = W3 @ spk2.T (2 passes), outputs ============
            c3_ps = ps.tile([DOUT, S], F32, name="c3_ps", tag="ps")
            n = len(CH_H2)
            for hi in range(n):
                MM(c3_ps, w3h_t[hi], s2_t[hi], start=(hi == 0), stop=False)
                MM(c3_ps, w3l_t[hi], s2_t[hi], start=False, stop=(hi == n - 1))
            spk3_t = outp.tile([DOUT, S], F32, name="spk3_t", tag="spk3_t")
            c3b_t = outp.tile([DOUT, S], F32, name="c3b_t", tag="c3b_t")
            mem3_t = outp.tile([DOUT, S], F32, name="mem3_t", tag="mem3_t")
            nc.vector.tensor_scalar(spk3_t, c3_ps, b3_t, 0.3, OP.add, OP.is_gt)
            nc.vector.tensor_scalar(c3b_t, c3_ps, b3_t, None, OP.add)
            nc.vector.scalar_tensor_tensor(mem3_t, spk3_t, -0.3, c3b_t,
                                           OP.mult, OP.add)
            nc.sync.dma_start(out=os_[b, :, :], in_=spk3_t)
            nc.sync.dma_start(out=om_[b, :, :], in_=mem3_t)

    nc.finalize()
    return nc


_NC_CACHE = {}


def _get_nc(nb):
    if nb not in _NC_CACHE:
        _NC_CACHE[nb] = build_nc(nb)
    return _NC_CACHE[nb]


def make_in_maps(x, We, be, Wq, bq, Wk, bk, Wv, bv, W2, b2, W3, b3,
                 ncores=NCORES):
    x = np.ascontiguousarray(x, np.float32)
    if x.max() > 1.0:
        x = (x * np.float32(1.0 / 255.0)).astype(np.float32)
    wEh, wEl = _split(np.ascontiguousarray(We.T))
    wQh, wQl = _split(np.ascontiguousarray(Wq.T))
    wKh, wKl = _split(np.ascontiguousarray(Wk.T))
    wVh, wVl = _split(np.ascontiguousarray(Wv.T))
    w2h, w2l = _split(np.ascontiguousarray(W2.T))
    w3h, w3l = _split(np.ascontiguousarray(W3.T))
    shared = dict(
        wEh=wEh, wEl=wEl, wQh=wQh, wQl=wQl, wKh=wKh, wKl=wKl,
        wVh=wVh, wVl=wVl, w2h=w2h, w2l=w2l, w3h=w3h, w3l=w3l,
        bE=np.ascontiguousarray(be.reshape(-1, 1), np.float32),
        bQ=np.ascontiguousarray(bq.reshape(-1, 1), np.float32),
        bK=np.ascontiguousarray(bk.reshape(-1, 1), np.float32),
        bV=np.ascontiguousarray(bv.reshape(-1, 1), np.float32),
        b2=np.ascontiguousarray(b2.reshape(-1, 1), np.float32),
        b3=np.ascontiguousarray(b3.reshape(-1, 1), np.float32),
        ident=np.eye(128, dtype=np.float32),
    )
    nb = x.shape[0] // ncores
    in_maps = []
    for c in range(ncores):
        xs = x[c * nb:(c + 1) * nb]                       # [nb, S, DIN]
        xT = np.ascontiguousarray(xs.transpose(0, 2, 1))  # [nb, DIN, S]
        xh_, xl_ = _split(xT)
        in_maps.append(dict(shared, xh=xh_, xl=xl_))
    return in_maps, nb


def kernel(x, We, be, Wq, bq, Wk, bk, Wv, bv, W2, b2, W3, b3, _trace=False):
    in_maps, nb = make_in_maps(x, We, be, Wq, bq, Wk, bk, Wv, bv, W2, b2, W3, b3)
    nc = _get_nc(nb)
    res = run_bass_kernel_spmd(nc, in_maps, list(range(NCORES)), trace=_trace)
    spk3 = np.concatenate([r["os"].transpose(0, 2, 1) for r in res.results], 0)
    mem3 = np.concatenate([r["om"].transpose(0, 2, 1) for r in res.results], 0)
    kernel.last_results = res
    return (np.ascontiguousarray(spk3, np.float32),
            np.ascontiguousarray(mem3, np.float32))
